# revision 21
# baseline (speedup 1.0000x reference)
"""Trainium2 Bass kernel for nn_EqvSelfAttention (B=4, N=1024, D=256, H=8).

Sharding: data-parallel over (batch b, query-half) -> 8 cores.
Each core computes all 8 heads for its 512 query rows against all 1024 keys.

Transfer-optimized: total host<->device traffic is ~17.4MB vs ~95.5MB for
the fp32 baseline (the measured window is dominated by input DMA, not
compute).
  * X_pairs ships as fp8 e3m4 (1.57MB/core), upcast to fp16 on device.
  * Y ships as fp16 halves (0.26MB/core); the full Y[b] is assembled on
    device by a pair-wise AllGather, and Q^T/V^T for "my" 512 queries are
    selected from both computed halves via a shipped 0/1 column (shl).
  * Weights (identical on all cores) ship as 1/8-shards (65KB/core) and
    are assembled by an 8-way AllGather; the structured constant matrices
    the fp32 baseline shipped (block-diag layer-1 'bd' 0.4MB, signed
    slot-collapse 'lr' 2.1MB) are built on device from ~100 scalars via
    affine_select masks + 32 tiny DMAs.
  * Output returns fp16 and is upcast on host.
Compute runs in fp16 on the PE (1 cycle/row vs 4 for fp32), accumulation
stays fp32 in PSUM; softmax reciprocal + final blend stay fp32.
Measured: rel err 2.9e-3 (gate 2e-2), device exec ~375us/core.

Math notes (vs reference):
  * 1/sqrt(D)=1/16 folded into Wq (exact power of two).
  * Per-head location-bias MLP: loc_h = sum_s wg2[h,s]*relu(hid_hs) + bg2[h].
    - |wg2| folded into layer-1 weights/bias; sign applied by the signed
      slot-collapse matmul that accumulates loc onto the content logits in
      PSUM (transposed layout [key, query]).
    - bg2 dropped: constant across keys => softmax-invariant.
  * Softmax without max subtraction (max logit ~7, exp fits fp16/fp32).
    Key presence folded into V'' = [pk*V | pk]; column 33 of A@V'' is the
    softmax denominator Z.
  * Absent queries (pq=0) attend uniformly in the reference => Oh = mean(V);
    blended in via (1-pq)*mean(V) at finalize.
"""

import sys
import numpy as np

sys.path.insert(0, "/opt/trn_rl_repo")

B, N, D, H, DH = 4, 1024, 256, 8, 32
R = 512  # query rows per core
NCORES = 8

_CACHE = {}


def _build_program():
    from contextlib import ExitStack

    from concourse import bass, mybir
    import concourse.tile as tile
    from concourse.masks import make_identity

    f32 = mybir.dt.float32
    f16 = mybir.dt.float16
    f8 = mybir.dt.float8e3
    AF = mybir.ActivationFunctionType
    OP = mybir.AluOpType
    ds = bass.ds

    nc = bass.Bass("TRN2", target_bir_lowering=False, debug=False, num_devices=8)

    # ---- I/O declarations (order matters for the PJRT call) ----
    # Each core ships only its 512-row half of Y[b]; the full Y is
    # assembled on device via a pair-wise AllGather (cores 2b, 2b+1).
    d_yh = nc.declare_dram_parameter("yh", [R, D], f16, isOutput=False)
    d_xp = nc.declare_dram_parameter("xp", [R, 3 * N], f8, isOutput=False)
    # weights are identical on all cores: each ships a 128-row shard of the
    # stacked [Wq; Wk; Wv; Wo] and the full matrix is AllGather-ed on device
    d_wh = nc.declare_dram_parameter("wh", [D // 2, D], f16, isOutput=False)
    d_b = nc.declare_dram_parameter("b", [4, D], f16, isOutput=False)
    d_bdc = nc.declare_dram_parameter("bdc", [3, 4 * H], f16, isOutput=False)
    d_rb = nc.declare_dram_parameter("rb", [128, H], f32, isOutput=False)
    d_sgn = nc.declare_dram_parameter("sgn", [128, H], f32, isOutput=False)
    d_pkc = nc.declare_dram_parameter("pkc", [128, 8], f32, isOutput=False)
    d_pq2 = nc.declare_dram_parameter("pq2", [2, R], f32, isOutput=False)
    d_shl = nc.declare_dram_parameter("shl", [128, 2], f32, isOutput=False)
    d_o = nc.declare_dram_parameter("o", [R, D], f16, isOutput=True)

    with tile.TileContext(nc) as tc:
        with ExitStack() as ctx:
            consts = ctx.enter_context(tc.tile_pool(name="consts", bufs=1))
            persist = ctx.enter_context(tc.tile_pool(name="persist", bufs=1))
            dram = ctx.enter_context(tc.tile_pool(name="dram", bufs=1, space="DRAM"))

            # ---------- constants ----------
            ident = consts.tile([128, 128], f16)
            make_identity(nc, ident)
            ident32 = consts.tile([128, 128], f32)
            make_identity(nc, ident32)
            ident8 = consts.tile([128, 128], f8)
            make_identity(nc, ident8)
            ones512 = consts.tile([1, 512], f16)
            nc.vector.memset(ones512, 1.0)
            ones128r = consts.tile([1, 128], f16)
            nc.vector.memset(ones128r, 1.0)
            ones128r32 = consts.tile([1, 128], f32)
            nc.vector.memset(ones128r32, 1.0)
            inv1024c = consts.tile([128, 1], f16)
            nc.vector.memset(inv1024c, 1.0 / 1024.0)

            # pair-wise AllGather of the Y halves first (phase A's PE work
            # starts with Y^T transposes, which need Y but not weights),
            # then the 8-way AllGather of the weight shards. DRAM bounce
            # buffers — collectives cannot touch I/O tensors directly.
            yin_b = dram.tile([R, D], f16)
            yg_b = dram.tile([N, D], f16)
            nc.gpsimd.dma_start(yin_b[:, :], d_yh[:, :])
            nc.gpsimd.collective_compute(
                "AllGather",
                mybir.AluOpType.bypass,
                replica_groups=[[0, 1], [2, 3], [4, 5], [6, 7]],
                ins=[yin_b.opt()],
                outs=[yg_b.opt()],
            )
            win_b = dram.tile([D // 2, D], f16)
            wg_b = dram.tile([4 * D, D], f16)
            nc.gpsimd.dma_start(win_b[:, :], d_wh[:, :])
            nc.gpsimd.collective_compute(
                "AllGather",
                mybir.AluOpType.bypass,
                replica_groups=[[0, 1, 2, 3, 4, 5, 6, 7]],
                ins=[win_b.opt()],
                outs=[wg_b.opt()],
            )

            # projection weights [in-part, tile, dout]
            wqs = consts.tile([128, 2, D], f16)
            nc.gpsimd.dma_start(wqs, wg_b[ds(0, D), :].rearrange("(t p) d -> p t d", p=128))
            wks = consts.tile([128, 2, D], f16)
            nc.gpsimd.dma_start(wks, wg_b[ds(D, D), :].rearrange("(t p) d -> p t d", p=128))
            wvs = consts.tile([128, 2, D], f16)
            nc.gpsimd.dma_start(wvs, wg_b[ds(2 * D, D), :].rearrange("(t p) d -> p t d", p=128))
            wos = consts.tile([128, 2, D], f16)
            nc.gpsimd.dma_start(wos, wg_b[ds(3 * D, D), :].rearrange("(t p) d -> p t d", p=128))
            # ---------- persistent activations ----------
            ktsb = persist.tile([128, 2, N], f16)      # K^T [dout, key]
            qtz = persist.tile([128, H, 512], f16)     # per-head zero-padded Q^T
            vsb = persist.tile([128, 8, D], f16)       # V [key, dout]
            v2sb = persist.tile([128, 8, H, 33], f16)  # [pk*V_h | pk]
            vtsb = persist.tile([128, 2, R], f32)      # V^T of my rows
            mvt = persist.tile([128, 2], f32)          # mean_k V (transposed col)
            xtall = persist.tile([96, 8, 4, 512], f16)  # Xp^T
            otsb = persist.tile([128, 2, R], f32)      # O^T accumulator
            ot16 = persist.tile([128, 2, R], f16)      # fp16 copy for phase C
            pqcb = persist.tile([128, R], f32)         # (1-pq) replicated rows

            nc.gpsimd.memset(qtz, 0.0)

            # ---------- phase B0: transpose X_pairs ----------
            with tc.tile_pool(name="xp_in", bufs=2) as xpin, \
                 tc.tile_pool(name="ps_t", bufs=2, space="PSUM") as pst:
                for kt in range(8):
                    xt_in = xpin.tile([128, 4, 384], f8)
                    for qt in range(4):
                        nc.sync.dma_start(
                            xt_in[:, qt],
                            d_xp[ds(128 * qt, 128), ds(384 * kt, 384)],
                        )
                    xt16 = xpin.tile([128, 4, 384], f16)
                    nc.scalar.copy(xt16, xt_in)
                    for cp in range(2):  # chunk pairs
                        ps = pst.tile([128, 2, 512], f16)
                        for ci in range(2):
                            for qt in range(4):
                                nc.tensor.transpose(
                                    ps[0:96, ci, ds(128 * qt, 128)],
                                    xt16[:, qt, ds(96 * (2 * cp + ci), 96)],
                                    ident,
                                )
                        if cp % 2 == 0:
                            nc.scalar.copy(
                                xtall[:, kt, ds(2 * cp, 2)], ps[0:96]
                            )
                        else:
                            nc.vector.tensor_copy(
                                xtall[:, kt, ds(2 * cp, 2)], ps[0:96]
                            )

            bqs = consts.tile([1, D], f16)
            nc.sync.dma_start(bqs, d_b[0:1, :])
            bks = consts.tile([1, D], f16)
            nc.sync.dma_start(bks, d_b[1:2, :])
            bvs = consts.tile([1, D], f16)
            nc.sync.dma_start(bvs, d_b[2:3, :])
            bos = consts.tile([1, D], f16)
            nc.sync.dma_start(bos, d_b[3:4, :])

            # block-diag layer-1 weights: bd[3k+c, h, 4k+s] = |wg2|*Wg1
            bdsb = consts.tile([96, H, 128], f16)
            nc.gpsimd.memset(bdsb, 0.0)
            bdc_view = d_bdc[:, :].rearrange("c (h s) -> c h s", s=4)
            for k in range(32):
                nc.sync.dma_start(bdsb[ds(3 * k, 3), :, ds(4 * k, 4)], bdc_view)

            rbsb = consts.tile([128, H], f32)
            nc.sync.dma_start(rbsb, d_rb[:, :])
            sgnsb = consts.tile([128, H], f32)
            nc.sync.dma_start(sgnsb, d_sgn[:, :])
            pkcs = consts.tile([128, 8], f32)
            nc.sync.dma_start(pkcs, d_pkc[:, :])
            pqs = consts.tile([1, R], f32)
            nc.sync.dma_start(pqs, d_pq2[0:1, :])
            pqcs = consts.tile([1, R], f32)
            nc.sync.dma_start(pqcs, d_pq2[1:2, :])
            shlsb = consts.tile([128, 2], f32)
            nc.sync.dma_start(shlsb, d_shl[:, :])

            # slot-collapse masks: msk[c4][p, j] = 1 iff p = 4*(j - 32*c4) + s,
            # s in {0,1,2}; then per-head signed copies msgn = sgn[p] * msk.
            msk = consts.tile([128, 4, 128], f16)
            nc.gpsimd.memset(msk, 1.0)
            for c4 in range(4):
                # s = p - 4j + 128*c4 ; keep where 0 <= s <= 2
                nc.gpsimd.affine_select(
                    out=msk[:, c4], in_=msk[:, c4],
                    compare_op=OP.is_ge, fill=0.0,
                    base=128 * c4, channel_multiplier=1, pattern=[[-4, 128]],
                )
                nc.gpsimd.affine_select(
                    out=msk[:, c4], in_=msk[:, c4],
                    compare_op=OP.is_ge, fill=0.0,
                    base=2 - 128 * c4, channel_multiplier=-1, pattern=[[4, 128]],
                )
            msgn = consts.tile([128, H, 4, 128], f16)
            for h in range(H):
                nc.vector.tensor_scalar(
                    msgn[:, h], msk, sgnsb[:, h : h + 1], None, op0=OP.mult
                )

            # ---------- phase A: Y^T and projections ----------
            with tc.tile_pool(name="ph_a", bufs=1) as pha, \
                 tc.tile_pool(name="ps_a", bufs=3, space="PSUM") as psa, \
                 tc.tile_pool(name="ps_at", bufs=2, space="PSUM") as psat:
                ysb = pha.tile([128, 8, D], f16)
                nc.gpsimd.dma_start(ysb, yg_b[:, :].rearrange("(t p) d -> p t d", p=128))

                yt = pha.tile([128, 2, N], f16)   # Y^T full batch
                for dt_ in range(2):
                    for g in range(2):  # groups of 4 n-tiles
                        ps = psat.tile([128, 512], f16)
                        for j in range(4):
                            nt = g * 4 + j
                            nc.tensor.transpose(
                                ps[:, ds(128 * j, 128)],
                                ysb[:, nt, ds(128 * dt_, 128)],
                                ident,
                            )
                        nc.vector.tensor_copy(yt[:, dt_, ds(512 * g, 512)], ps)

                qtsb = pha.tile([128, 2, R], f16)
                qh = pha.tile([128, 2, 512], f16)
                vh = pha.tile([128, 2, 512], f32)
                # Q^T (scaled Wq), K^T, V, V^T projections; Q^T/V^T are
                # computed for both query halves and my half is selected
                # via the shipped 0/1 column shl.
                for dt_ in range(2):
                    for half in range(2):
                        ps = psa.tile([128, 512], f32)
                        for k_ in range(2):
                            nc.tensor.matmul(
                                ps, wks[:, k_, ds(128 * dt_, 128)],
                                yt[:, k_, ds(512 * half, 512)],
                                start=(k_ == 0), stop=False,
                            )
                        nc.tensor.matmul(
                            ps, bks[0:1, ds(128 * dt_, 128)], ones512,
                            start=False, stop=True,
                        )
                        nc.vector.tensor_copy(ktsb[:, dt_, ds(512 * half, 512)], ps)

                        ps = psa.tile([128, 512], f32)
                        for k_ in range(2):
                            nc.tensor.matmul(
                                ps, wqs[:, k_, ds(128 * dt_, 128)],
                                yt[:, k_, ds(512 * half, 512)],
                                start=(k_ == 0), stop=False,
                            )
                        nc.tensor.matmul(
                            ps, bqs[0:1, ds(128 * dt_, 128)], ones512,
                            start=False, stop=True,
                        )
                        nc.vector.tensor_scalar(
                            qh[:, half], ps, shlsb[:, half : half + 1], None,
                            op0=OP.mult,
                        )

                        ps = psa.tile([128, 512], f32)
                        for k_ in range(2):
                            nc.tensor.matmul(
                                ps, wvs[:, k_, ds(128 * dt_, 128)],
                                yt[:, k_, ds(512 * half, 512)],
                                start=(k_ == 0), stop=False,
                            )
                        nc.tensor.matmul(
                            ps, bvs[0:1, ds(128 * dt_, 128)], ones512,
                            start=False, stop=True,
                        )
                        nc.vector.tensor_scalar(
                            vh[:, half], ps, shlsb[:, half : half + 1], None,
                            op0=OP.mult,
                        )
                    nc.vector.tensor_add(qtsb[:, dt_], qh[:, 0], qh[:, 1])
                    nc.vector.tensor_add(vtsb[:, dt_], vh[:, 0], vh[:, 1])

                for nt in range(8):
                    ps = psa.tile([128, 256], f32)
                    for k_ in range(2):
                        nc.tensor.matmul(
                            ps, yt[:, k_, ds(128 * nt, 128)], wvs[:, k_],
                            start=(k_ == 0), stop=False,
                        )
                    nc.tensor.matmul(ps, ones128r, bvs, start=False, stop=True)
                    nc.vector.tensor_copy(vsb[:, nt], ps)

                # per-head zero-padded Q^T slices (keeps content matmuls K=128)
                for h in range(H):
                    base = 32 * (h % 4)
                    nc.vector.tensor_copy(
                        qtz[ds(base, 32), h], qtsb[ds(base, 32), h // 4]
                    )

                # V'' = [pk * V_h | pk]
                for nt in range(8):
                    nc.vector.tensor_scalar(
                        v2sb[:, nt, :, 0:32],
                        vsb[:, nt].rearrange("p (h d) -> p h d", h=H),
                        pkcs[:, nt : nt + 1],
                        None,
                        op0=OP.mult,
                    )
                    nc.vector.tensor_copy(
                        v2sb[:, nt, :, 32:33],
                        pkcs[:, nt : nt + 1].to_broadcast((128, H, 1)),
                    )

                # mean_k V (transposed): mvt[d] = sum_n V[n, d] / 1024
                psmv = psa.tile([128, 2], f32)
                for dt_ in range(2):
                    for nt in range(8):
                        nc.tensor.matmul(
                            psmv[:, dt_ : dt_ + 1],
                            vsb[:, nt, ds(128 * dt_, 128)],
                            inv1024c,
                            start=(nt == 0), stop=(nt == 7),
                        )
                nc.vector.tensor_copy(mvt, psmv)

            # ---------- phase B1: attention main loop ----------
            with tc.tile_pool(name="ps_ct", bufs=2, space="PSUM") as psct, \
                 tc.tile_pool(name="ps_z", bufs=2, space="PSUM") as psz, \
                 tc.tile_pool(name="ps_av", bufs=2, space="PSUM") as psav, \
                 tc.tile_pool(name="rz_p", bufs=2) as rzp, \
                 tc.tile_pool(name="et_p", bufs=2) as etp, \
                 tc.tile_pool(name="fin_p", bufs=2) as finp:
                # replicate (1-pq) across partitions via a K=1 outer product
                psb = psct.tile([128, 512], f32, name="psbc", tag="ct")
                nc.tensor.matmul(psb, ones128r32, pqcs, start=True, stop=True)
                nc.vector.tensor_copy(pqcb, psb)
                def _finalize(h, av):
                    # finalize head h (weights fp32; broadcast runs fp16)
                    rec = finp.tile([1, 512], f32)
                    nc.vector.reciprocal(rec, av[32:33])
                    rpq = finp.tile([1, 512], f16)
                    nc.vector.tensor_mul(rpq, rec, pqs)
                    nc.tensor.matmul(
                        av[64:96], ones128r[0:1, 0:32], rpq, start=True, stop=True
                    )
                    rpqs = finp.tile([32, 512], f32)
                    nc.vector.tensor_copy(rpqs, av[64:96])
                    t2 = finp.tile([32, 512], f32)
                    nc.vector.tensor_mul(t2, av[0:32], rpqs)
                    mv0 = finp.tile([32, 1], f32)
                    nc.vector.tensor_copy(
                        mv0, mvt[ds(32 * (h % 4), 32), h // 4 : h // 4 + 1]
                    )
                    t3 = finp.tile([32, 512], f32)
                    nc.vector.tensor_scalar(
                        t3, pqcb[0:32], mv0, None, op0=OP.mult
                    )
                    t4 = finp.tile([32, 512], f32)
                    nc.vector.tensor_add(t4, t2, t3)
                    vt0 = finp.tile([32, 512], f32)
                    nc.vector.tensor_copy(vt0, vtsb[ds(32 * (h % 4), 32), h // 4])
                    nc.vector.tensor_add(
                        otsb[ds(32 * (h % 4), 32), h // 4], t4, vt0
                    )
                    nc.scalar.copy(
                        ot16[ds(32 * (h % 4), 32), h // 4],
                        otsb[ds(32 * (h % 4), 32), h // 4],
                    )

                avs = {}
                for h in range(H):
                    av = psav.tile([128, 512], f32)
                    avs[h] = av
                    for kt in range(8):
                        ct = psct.tile([128, 512], f32, name="ct", tag="ct")
                        nc.tensor.matmul(
                            ct,
                            ktsb[:, h // 4, ds(128 * kt, 128)],
                            qtz[:, h],
                            start=True, stop=False,
                        )
                        rzs = []
                        for cp in range(2):
                            zps = psz.tile([128, 2, 512], f32)
                            for ci in range(2):
                                nc.tensor.matmul(
                                    zps[:, ci], bdsb[:, h],
                                    xtall[:, kt, 2 * cp + ci],
                                    start=True, stop=True,
                                )
                            rz = rzp.tile([128, 2, 512], f16)
                            if cp % 2 == 0:
                                nc.scalar.activation(
                                    rz, zps, AF.Relu, bias=rbsb[:, h : h + 1]
                                )
                            else:
                                nc.vector.tensor_scalar(
                                    rz, zps, rbsb[:, h : h + 1], 0.0,
                                    op0=OP.add, op1=OP.max,
                                )
                            rzs.append(rz)
                        for c4 in range(4):
                            nc.tensor.matmul(
                                ct, msgn[:, h, c4], rzs[c4 // 2][:, c4 % 2],
                                start=False, stop=(c4 == 3),
                            )
                        et = etp.tile([128, 512], f16)
                        nc.scalar.activation(et, ct, AF.Exp)
                        nc.tensor.matmul(
                            av[0:33], v2sb[:, kt, h], et,
                            start=(kt == 0), stop=(kt == 7),
                        )

                    if h > 0:
                        _finalize(h - 1, avs[h - 1])
                _finalize(H - 1, avs[H - 1])
            # ---------- phase C: O = O + relu(O @ Wo + bo) ----------
            with tc.tile_pool(name="ps_o", bufs=2, space="PSUM") as pso, \
                 tc.tile_pool(name="o_p", bufs=2) as op_:
                for j in range(4):
                    pso1 = pso.tile([128, 256], f32)
                    for dt_ in range(2):
                        nc.tensor.transpose(
                            pso1[:, ds(128 * dt_, 128)],
                            otsb[:, dt_, ds(128 * j, 128)],
                            ident32,
                        )
                    oj = op_.tile([128, 256], f32)
                    nc.vector.tensor_copy(oj, pso1)

                    pso2 = pso.tile([128, 256], f32)
                    for dt_ in range(2):
                        nc.tensor.matmul(
                            pso2, ot16[:, dt_, ds(128 * j, 128)], wos[:, dt_],
                            start=(dt_ == 0), stop=False,
                        )
                    nc.tensor.matmul(pso2, ones128r, bos, start=False, stop=True)
                    r2 = op_.tile([128, 256], f32)
                    nc.scalar.activation(r2, pso2, AF.Relu)
                    ofin = op_.tile([128, 256], f16)
                    nc.vector.tensor_add(ofin, oj, r2)
                    nc.sync.dma_start(d_o[ds(128 * j, 128), :], ofin)

    _split_multiwait(nc, mybir)
    return nc


def _split_multiwait(nc, mybir):
    """This walrus build only encodes ONE sem-wait per instruction; Tile's
    tail drain carries several. Split extras onto preceding NoOps."""
    for f in nc.m.functions:
        for blk in f.blocks:
            insts = list(blk.instructions)
            changed = False
            newlist = []
            for ins in insts:
                si = ins.sync_info
                if si is not None and len(si.on_wait) > 1:
                    waits = list(si.on_wait)
                    for j, w in enumerate(waits[:-1]):
                        newlist.append(
                            mybir.InstNoOp(
                                name=f"{ins.name}_splitw{j}",
                                engine=ins.engine,
                                ins=[],
                                outs=[],
                                sync_info=mybir.SyncInfo(on_wait=[w], on_update=[]),
                            )
                        )
                    ins.sync_info = mybir.SyncInfo(
                        on_wait=[waits[-1]], on_update=list(si.on_update)
                    )
                    changed = True
                newlist.append(ins)
            if changed:
                blk.instructions = newlist


def _host_constants(Wg1, bg1, wg2, bg2):
    """Compact folded layer-1 table, relu bias column, sign column."""
    aw = np.abs(wg2)  # [H, 3]
    sw = np.sign(wg2).astype(np.float32)
    kk = np.arange(32)

    # bdc[c, 4h+s] = |wg2[h,s]| * Wg1[h,c,s]
    bdc = np.zeros((3, 4 * H), np.float16)
    for c in range(3):
        for s in range(3):
            bdc[c, 4 * np.arange(H) + s] = (aw[:, s] * Wg1[:, c, s]).astype(np.float16)
    rb = np.zeros((128, H), np.float32)
    sgn = np.zeros((128, H), np.float32)
    for s in range(3):
        rb[4 * kk + s, :] = (aw[:, s] * bg1[:, s])[np.newaxis, :]
        sgn[4 * kk + s, :] = sw[np.newaxis, :, s]
    return bdc, rb, sgn


def _cast_threaded(src, dtype, nthreads=8):
    """dtype-cast in chunks on a thread pool (ml_dtypes casts release the
    GIL; harmless on a single-cpu host)."""
    from concurrent.futures import ThreadPoolExecutor

    flat = src.reshape(-1)
    out = np.empty(flat.shape, dtype)
    bounds = np.linspace(0, flat.size, nthreads + 1).astype(np.int64)

    def chunk(i):
        out[bounds[i]:bounds[i + 1]] = flat[bounds[i]:bounds[i + 1]]

    with ThreadPoolExecutor(nthreads) as ex:
        list(ex.map(chunk, range(nthreads)))
    return out.reshape(src.shape)


def kernel(**inputs):
    import ml_dtypes
    from concourse.bass_utils import run_bass_kernel_spmd

    Y = np.asarray(inputs["Y_lift"], np.float32)
    XP = np.asarray(inputs["X_pairs"], np.float32)
    PQ = np.asarray(inputs["presence_q"], np.float32)
    PK = np.asarray(inputs["presence_k"], np.float32)

    bdc, rb, sgn = _host_constants(
        np.asarray(inputs["Wg1"], np.float32),
        np.asarray(inputs["bg1"], np.float32),
        np.asarray(inputs["wg2"], np.float32),
        np.asarray(inputs["bg2"], np.float32),
    )

    # one bulk cast each; per-core slices below are zero-copy views
    XP8 = _cast_threaded(
        XP.reshape(B, N, 3 * N), ml_dtypes.float8_e3m4
    )
    Y16 = Y.astype(np.float16)
    w16 = np.concatenate(
        [
            np.asarray(inputs["Wq"], np.float32) / 16.0,
            np.asarray(inputs["Wk"], np.float32),
            np.asarray(inputs["Wv"], np.float32),
            np.asarray(inputs["Wo"], np.float32),
        ],
        axis=0,
    ).astype(np.float16)
    b16 = np.stack(
        [
            np.asarray(inputs["bq"], np.float32) / 16.0,
            np.asarray(inputs["bk"], np.float32),
            np.asarray(inputs["bv"], np.float32),
            np.asarray(inputs["bo"], np.float32),
        ],
        axis=0,
    ).astype(np.float16)

    if "nc" not in _CACHE:
        _CACHE["nc"] = _build_program()
    nc = _CACHE["nc"]

    shl = [np.zeros((128, 2), np.float32) for _ in range(2)]
    shl[0][:, 0] = 1.0
    shl[1][:, 1] = 1.0

    in_maps = []
    for core in range(NCORES):
        b, half = core // 2, core % 2
        rows = slice(half * R, half * R + R)
        pq2 = np.empty((2, R), np.float32)
        pq2[0] = PQ[b, rows]
        pq2[1] = 1.0 - PQ[b, rows]
        in_maps.append(
            {
                "yh": Y16[b, rows],
                "xp": XP8[b, rows],
                "wh": w16[128 * core : 128 * (core + 1)],
                "b": b16,
                "bdc": bdc,
                "rb": rb,
                "sgn": sgn,
                "pkc": np.ascontiguousarray(PK[b].reshape(8, 128).T),
                "pq2": pq2,
                "shl": shl[half],
            }
        )

    res = run_bass_kernel_spmd(nc, in_maps, core_ids=list(range(NCORES)))
    out = np.empty((B, N, D), np.float32)
    for core in range(NCORES):
        b, half = core // 2, core % 2
        out[b, half * R : half * R + R] = res.results[core]["o"]
    return out


# revision 22
# speedup vs baseline: 1.3674x; 1.3674x over previous
"""Trainium2 Bass kernel for nn_EqvSelfAttention (B=4, N=1024, D=256, H=8).

Sharding: data-parallel over (batch b, query-half) -> 8 cores.
Each core computes all 8 heads for its 512 query rows against all 1024 keys.

Transfer-optimized: total host<->device traffic is ~17.4MB vs ~95.5MB for
the fp32 baseline (the measured window is dominated by input DMA, not
compute).
  * X_pairs ships as fp8 e3m4 (1.57MB/core), upcast to fp16 on device.
  * Y ships as fp16 halves (0.26MB/core); the full Y[b] is assembled on
    device by a pair-wise AllGather, and Q^T/V^T for "my" 512 queries are
    selected from both computed halves via a shipped 0/1 column (shl).
  * Weights (identical on all cores) ship as 1/8-shards (65KB/core) and
    are assembled by an 8-way AllGather; the structured constant matrices
    the fp32 baseline shipped (block-diag layer-1 'bd' 0.4MB, signed
    slot-collapse 'lr' 2.1MB) are built on device from ~100 scalars via
    affine_select masks + 32 tiny DMAs.
  * Output returns fp16 and is upcast on host.
Compute runs in fp16 on the PE (1 cycle/row vs 4 for fp32), accumulation
stays fp32 in PSUM; softmax reciprocal + final blend stay fp32.
Measured: rel err 2.9e-3 (gate 2e-2), device exec ~375us/core.

Math notes (vs reference):
  * 1/sqrt(D)=1/16 folded into Wq (exact power of two).
  * Per-head location-bias MLP: loc_h = sum_s wg2[h,s]*relu(hid_hs) + bg2[h].
    - |wg2| folded into layer-1 weights/bias; sign applied by the signed
      slot-collapse matmul that accumulates loc onto the content logits in
      PSUM (transposed layout [key, query]).
    - bg2 dropped: constant across keys => softmax-invariant.
  * Softmax without max subtraction (max logit ~7, exp fits fp16/fp32).
    Key presence folded into V'' = [pk*V | pk]; column 33 of A@V'' is the
    softmax denominator Z.
  * Absent queries (pq=0) attend uniformly in the reference => Oh = mean(V);
    blended in via (1-pq)*mean(V) at finalize.
"""

import sys
import numpy as np

sys.path.insert(0, "/opt/trn_rl_repo")

B, N, D, H, DH = 4, 1024, 256, 8, 32
R = 512  # query rows per core
NCORES = 8

_CACHE = {}


def _build_program():
    from contextlib import ExitStack

    from concourse import bass, mybir
    import concourse.tile as tile
    from concourse.masks import make_identity

    f32 = mybir.dt.float32
    f16 = mybir.dt.float16
    f8 = mybir.dt.float8e3
    AF = mybir.ActivationFunctionType
    OP = mybir.AluOpType
    ds = bass.ds

    nc = bass.Bass("TRN2", target_bir_lowering=False, debug=False, num_devices=8)

    # ---- I/O declarations (order matters for the PJRT call) ----
    # Each core ships only its 512-row half of Y[b]; the full Y is
    # assembled on device via a pair-wise AllGather (cores 2b, 2b+1).
    d_yh = nc.declare_dram_parameter("yh", [R, D], f16, isOutput=False)
    d_xp = nc.declare_dram_parameter("xp", [R, 3 * N], f8, isOutput=False)
    # weights are identical on all cores: each ships a 128-row shard of the
    # stacked [Wq; Wk; Wv; Wo] and the full matrix is AllGather-ed on device
    d_wh = nc.declare_dram_parameter("wh", [D // 2, D], f16, isOutput=False)
    d_b = nc.declare_dram_parameter("b", [4, D], f16, isOutput=False)
    d_bdc = nc.declare_dram_parameter("bdc", [3, 4 * H], f16, isOutput=False)
    d_rb = nc.declare_dram_parameter("rb", [128, H], f32, isOutput=False)
    d_sgn = nc.declare_dram_parameter("sgn", [128, H], f32, isOutput=False)
    d_pkc = nc.declare_dram_parameter("pkc", [128, 8], f32, isOutput=False)
    d_pq2 = nc.declare_dram_parameter("pq2", [2, R], f32, isOutput=False)
    d_shl = nc.declare_dram_parameter("shl", [128, 2], f32, isOutput=False)
    d_o = nc.declare_dram_parameter("o", [R, D], f16, isOutput=True)

    with tile.TileContext(nc) as tc:
        with ExitStack() as ctx:
            consts = ctx.enter_context(tc.tile_pool(name="consts", bufs=1))
            persist = ctx.enter_context(tc.tile_pool(name="persist", bufs=1))
            dram = ctx.enter_context(tc.tile_pool(name="dram", bufs=1, space="DRAM"))

            # ---------- constants ----------
            ident = consts.tile([128, 128], f16)
            make_identity(nc, ident)
            ident32 = consts.tile([128, 128], f32)
            make_identity(nc, ident32)
            ident8 = consts.tile([128, 128], f8)
            make_identity(nc, ident8)
            ones512 = consts.tile([1, 512], f16)
            nc.vector.memset(ones512, 1.0)
            ones128r = consts.tile([1, 128], f16)
            nc.vector.memset(ones128r, 1.0)
            ones128r32 = consts.tile([1, 128], f32)
            nc.vector.memset(ones128r32, 1.0)
            inv1024c = consts.tile([128, 1], f16)
            nc.vector.memset(inv1024c, 1.0 / 1024.0)

            # pair-wise AllGather of the Y halves first (phase A's PE work
            # starts with Y^T transposes, which need Y but not weights),
            # then the 8-way AllGather of the weight shards. DRAM bounce
            # buffers — collectives cannot touch I/O tensors directly.
            yin_b = dram.tile([R, D], f16)
            yg_b = dram.tile([N, D], f16)
            nc.gpsimd.dma_start(yin_b[:, :], d_yh[:, :])
            nc.gpsimd.collective_compute(
                "AllGather",
                mybir.AluOpType.bypass,
                replica_groups=[[0, 1], [2, 3], [4, 5], [6, 7]],
                ins=[yin_b.opt()],
                outs=[yg_b.opt()],
            )
            win_b = dram.tile([D // 2, D], f16)
            wg_b = dram.tile([4 * D, D], f16)
            nc.gpsimd.dma_start(win_b[:, :], d_wh[:, :])
            nc.gpsimd.collective_compute(
                "AllGather",
                mybir.AluOpType.bypass,
                replica_groups=[[0, 1, 2, 3, 4, 5, 6, 7]],
                ins=[win_b.opt()],
                outs=[wg_b.opt()],
            )

            # projection weights [in-part, tile, dout]
            wqs = consts.tile([128, 2, D], f16)
            nc.gpsimd.dma_start(wqs, wg_b[ds(0, D), :].rearrange("(t p) d -> p t d", p=128))
            wks = consts.tile([128, 2, D], f16)
            nc.gpsimd.dma_start(wks, wg_b[ds(D, D), :].rearrange("(t p) d -> p t d", p=128))
            wvs = consts.tile([128, 2, D], f16)
            nc.gpsimd.dma_start(wvs, wg_b[ds(2 * D, D), :].rearrange("(t p) d -> p t d", p=128))
            wos = consts.tile([128, 2, D], f16)
            nc.gpsimd.dma_start(wos, wg_b[ds(3 * D, D), :].rearrange("(t p) d -> p t d", p=128))
            bqs = consts.tile([1, D], f16)
            nc.sync.dma_start(bqs, d_b[0:1, :])
            bks = consts.tile([1, D], f16)
            nc.sync.dma_start(bks, d_b[1:2, :])
            bvs = consts.tile([1, D], f16)
            nc.sync.dma_start(bvs, d_b[2:3, :])
            bos = consts.tile([1, D], f16)
            nc.sync.dma_start(bos, d_b[3:4, :])

            # block-diag layer-1 weights: bd[3k+c, h, 4k+s] = |wg2|*Wg1
            bdsb = consts.tile([96, H, 128], f16)
            nc.gpsimd.memset(bdsb, 0.0)
            bdc_view = d_bdc[:, :].rearrange("c (h s) -> c h s", s=4)
            for k in range(32):
                nc.sync.dma_start(bdsb[ds(3 * k, 3), :, ds(4 * k, 4)], bdc_view)

            rbsb = consts.tile([128, H], f32)
            nc.sync.dma_start(rbsb, d_rb[:, :])
            sgnsb = consts.tile([128, H], f32)
            nc.sync.dma_start(sgnsb, d_sgn[:, :])
            pkcs = consts.tile([128, 8], f32)
            nc.sync.dma_start(pkcs, d_pkc[:, :])
            pqs = consts.tile([1, R], f32)
            nc.sync.dma_start(pqs, d_pq2[0:1, :])
            pqcs = consts.tile([1, R], f32)
            nc.sync.dma_start(pqcs, d_pq2[1:2, :])
            shlsb = consts.tile([128, 2], f32)
            nc.sync.dma_start(shlsb, d_shl[:, :])

            # slot-collapse masks: msk[c4][p, j] = 1 iff p = 4*(j - 32*c4) + s,
            # s in {0,1,2}; then per-head signed copies msgn = sgn[p] * msk.
            msk = consts.tile([128, 4, 128], f16)
            nc.gpsimd.memset(msk, 1.0)
            for c4 in range(4):
                # s = p - 4j + 128*c4 ; keep where 0 <= s <= 2
                nc.gpsimd.affine_select(
                    out=msk[:, c4], in_=msk[:, c4],
                    compare_op=OP.is_ge, fill=0.0,
                    base=128 * c4, channel_multiplier=1, pattern=[[-4, 128]],
                )
                nc.gpsimd.affine_select(
                    out=msk[:, c4], in_=msk[:, c4],
                    compare_op=OP.is_ge, fill=0.0,
                    base=2 - 128 * c4, channel_multiplier=-1, pattern=[[4, 128]],
                )
            msgn = consts.tile([128, H, 4, 128], f16)
            for h in range(H):
                nc.vector.tensor_scalar(
                    msgn[:, h], msk, sgnsb[:, h : h + 1], None, op0=OP.mult
                )

            # ---------- persistent activations ----------
            ktsb = persist.tile([128, 2, N], f16)      # K^T [dout, key]
            qtz = persist.tile([128, H, 512], f16)     # per-head zero-padded Q^T
            vsb = persist.tile([128, 8, D], f16)       # V [key, dout]
            v2sb = persist.tile([128, 8, H, 33], f16)  # [pk*V_h | pk]
            vtsb = persist.tile([128, 2, R], f32)      # V^T of my rows
            mvt = persist.tile([128, 2], f32)          # mean_k V (transposed col)
            xtall = persist.tile([96, 8, 4, 512], f16)  # Xp^T
            otsb = persist.tile([128, 2, R], f32)      # O^T accumulator
            ot16 = persist.tile([128, 2, R], f16)      # fp16 copy for phase C
            pqcb = persist.tile([128, R], f32)         # (1-pq) replicated rows

            nc.gpsimd.memset(qtz, 0.0)

            # ---------- phase B0: transpose X_pairs ----------
            with tc.tile_pool(name="xp_in", bufs=2) as xpin, \
                 tc.tile_pool(name="ps_t", bufs=2, space="PSUM") as pst:
                for kt in range(8):
                    xt_in = xpin.tile([128, 4, 384], f8)
                    for qt in range(4):
                        nc.sync.dma_start(
                            xt_in[:, qt],
                            d_xp[ds(128 * qt, 128), ds(384 * kt, 384)],
                        )
                    xt16 = xpin.tile([128, 4, 384], f16)
                    nc.scalar.copy(xt16, xt_in)
                    for cp in range(2):  # chunk pairs
                        ps = pst.tile([128, 2, 512], f16)
                        for ci in range(2):
                            for qt in range(4):
                                nc.tensor.transpose(
                                    ps[0:96, ci, ds(128 * qt, 128)],
                                    xt16[:, qt, ds(96 * (2 * cp + ci), 96)],
                                    ident,
                                )
                        if cp % 2 == 0:
                            nc.scalar.copy(
                                xtall[:, kt, ds(2 * cp, 2)], ps[0:96]
                            )
                        else:
                            nc.vector.tensor_copy(
                                xtall[:, kt, ds(2 * cp, 2)], ps[0:96]
                            )

            # ---------- phase A: Y^T and projections ----------
            with tc.tile_pool(name="ph_a", bufs=1) as pha, \
                 tc.tile_pool(name="ps_a", bufs=3, space="PSUM") as psa, \
                 tc.tile_pool(name="ps_at", bufs=2, space="PSUM") as psat:
                ysb = pha.tile([128, 8, D], f16)
                nc.gpsimd.dma_start(ysb, yg_b[:, :].rearrange("(t p) d -> p t d", p=128))

                yt = pha.tile([128, 2, N], f16)   # Y^T full batch
                for dt_ in range(2):
                    for g in range(2):  # groups of 4 n-tiles
                        ps = psat.tile([128, 512], f16)
                        for j in range(4):
                            nt = g * 4 + j
                            nc.tensor.transpose(
                                ps[:, ds(128 * j, 128)],
                                ysb[:, nt, ds(128 * dt_, 128)],
                                ident,
                            )
                        nc.vector.tensor_copy(yt[:, dt_, ds(512 * g, 512)], ps)

                qtsb = pha.tile([128, 2, R], f16)
                qh = pha.tile([128, 2, 512], f16)
                vh = pha.tile([128, 2, 512], f32)
                # Q^T (scaled Wq), K^T, V, V^T projections; Q^T/V^T are
                # computed for both query halves and my half is selected
                # via the shipped 0/1 column shl.
                for dt_ in range(2):
                    for half in range(2):
                        ps = psa.tile([128, 512], f32)
                        for k_ in range(2):
                            nc.tensor.matmul(
                                ps, wks[:, k_, ds(128 * dt_, 128)],
                                yt[:, k_, ds(512 * half, 512)],
                                start=(k_ == 0), stop=False,
                            )
                        nc.tensor.matmul(
                            ps, bks[0:1, ds(128 * dt_, 128)], ones512,
                            start=False, stop=True,
                        )
                        nc.vector.tensor_copy(ktsb[:, dt_, ds(512 * half, 512)], ps)

                        ps = psa.tile([128, 512], f32)
                        for k_ in range(2):
                            nc.tensor.matmul(
                                ps, wqs[:, k_, ds(128 * dt_, 128)],
                                yt[:, k_, ds(512 * half, 512)],
                                start=(k_ == 0), stop=False,
                            )
                        nc.tensor.matmul(
                            ps, bqs[0:1, ds(128 * dt_, 128)], ones512,
                            start=False, stop=True,
                        )
                        nc.vector.tensor_scalar(
                            qh[:, half], ps, shlsb[:, half : half + 1], None,
                            op0=OP.mult,
                        )

                        ps = psa.tile([128, 512], f32)
                        for k_ in range(2):
                            nc.tensor.matmul(
                                ps, wvs[:, k_, ds(128 * dt_, 128)],
                                yt[:, k_, ds(512 * half, 512)],
                                start=(k_ == 0), stop=False,
                            )
                        nc.tensor.matmul(
                            ps, bvs[0:1, ds(128 * dt_, 128)], ones512,
                            start=False, stop=True,
                        )
                        nc.vector.tensor_scalar(
                            vh[:, half], ps, shlsb[:, half : half + 1], None,
                            op0=OP.mult,
                        )
                    nc.vector.tensor_add(qtsb[:, dt_], qh[:, 0], qh[:, 1])
                    nc.vector.tensor_add(vtsb[:, dt_], vh[:, 0], vh[:, 1])

                for nt in range(8):
                    ps = psa.tile([128, 256], f32)
                    for k_ in range(2):
                        nc.tensor.matmul(
                            ps, yt[:, k_, ds(128 * nt, 128)], wvs[:, k_],
                            start=(k_ == 0), stop=False,
                        )
                    nc.tensor.matmul(ps, ones128r, bvs, start=False, stop=True)
                    nc.vector.tensor_copy(vsb[:, nt], ps)

                # per-head zero-padded Q^T slices (keeps content matmuls K=128)
                for h in range(H):
                    base = 32 * (h % 4)
                    nc.vector.tensor_copy(
                        qtz[ds(base, 32), h], qtsb[ds(base, 32), h // 4]
                    )

                # V'' = [pk * V_h | pk]
                for nt in range(8):
                    nc.vector.tensor_scalar(
                        v2sb[:, nt, :, 0:32],
                        vsb[:, nt].rearrange("p (h d) -> p h d", h=H),
                        pkcs[:, nt : nt + 1],
                        None,
                        op0=OP.mult,
                    )
                    nc.vector.tensor_copy(
                        v2sb[:, nt, :, 32:33],
                        pkcs[:, nt : nt + 1].to_broadcast((128, H, 1)),
                    )

                # mean_k V (transposed): mvt[d] = sum_n V[n, d] / 1024
                psmv = psa.tile([128, 2], f32)
                for dt_ in range(2):
                    for nt in range(8):
                        nc.tensor.matmul(
                            psmv[:, dt_ : dt_ + 1],
                            vsb[:, nt, ds(128 * dt_, 128)],
                            inv1024c,
                            start=(nt == 0), stop=(nt == 7),
                        )
                nc.vector.tensor_copy(mvt, psmv)

            # ---------- phase B1: attention main loop ----------
            with tc.tile_pool(name="ps_ct", bufs=2, space="PSUM") as psct, \
                 tc.tile_pool(name="ps_z", bufs=2, space="PSUM") as psz, \
                 tc.tile_pool(name="ps_av", bufs=2, space="PSUM") as psav, \
                 tc.tile_pool(name="rz_p", bufs=2) as rzp, \
                 tc.tile_pool(name="et_p", bufs=2) as etp, \
                 tc.tile_pool(name="fin_p", bufs=2) as finp:
                # replicate (1-pq) across partitions via a K=1 outer product
                psb = psct.tile([128, 512], f32, name="psbc", tag="ct")
                nc.tensor.matmul(psb, ones128r32, pqcs, start=True, stop=True)
                nc.vector.tensor_copy(pqcb, psb)
                def _finalize(h, av):
                    # finalize head h (weights fp32; broadcast runs fp16)
                    rec = finp.tile([1, 512], f32)
                    nc.vector.reciprocal(rec, av[32:33])
                    rpq = finp.tile([1, 512], f16)
                    nc.vector.tensor_mul(rpq, rec, pqs)
                    nc.tensor.matmul(
                        av[64:96], ones128r[0:1, 0:32], rpq, start=True, stop=True
                    )
                    rpqs = finp.tile([32, 512], f32)
                    nc.vector.tensor_copy(rpqs, av[64:96])
                    t2 = finp.tile([32, 512], f32)
                    nc.vector.tensor_mul(t2, av[0:32], rpqs)
                    mv0 = finp.tile([32, 1], f32)
                    nc.vector.tensor_copy(
                        mv0, mvt[ds(32 * (h % 4), 32), h // 4 : h // 4 + 1]
                    )
                    t3 = finp.tile([32, 512], f32)
                    nc.vector.tensor_scalar(
                        t3, pqcb[0:32], mv0, None, op0=OP.mult
                    )
                    t4 = finp.tile([32, 512], f32)
                    nc.vector.tensor_add(t4, t2, t3)
                    vt0 = finp.tile([32, 512], f32)
                    nc.vector.tensor_copy(vt0, vtsb[ds(32 * (h % 4), 32), h // 4])
                    nc.vector.tensor_add(
                        otsb[ds(32 * (h % 4), 32), h // 4], t4, vt0
                    )
                    nc.scalar.copy(
                        ot16[ds(32 * (h % 4), 32), h // 4],
                        otsb[ds(32 * (h % 4), 32), h // 4],
                    )

                avs = {}
                for h in range(H):
                    av = psav.tile([128, 512], f32)
                    avs[h] = av
                    for kt in range(8):
                        ct = psct.tile([128, 512], f32, name="ct", tag="ct")
                        nc.tensor.matmul(
                            ct,
                            ktsb[:, h // 4, ds(128 * kt, 128)],
                            qtz[:, h],
                            start=True, stop=False,
                        )
                        rzs = []
                        for cp in range(2):
                            zps = psz.tile([128, 2, 512], f32)
                            for ci in range(2):
                                nc.tensor.matmul(
                                    zps[:, ci], bdsb[:, h],
                                    xtall[:, kt, 2 * cp + ci],
                                    start=True, stop=True,
                                )
                            rz = rzp.tile([128, 2, 512], f16)
                            if cp % 2 == 0:
                                nc.scalar.activation(
                                    rz, zps, AF.Relu, bias=rbsb[:, h : h + 1]
                                )
                            else:
                                nc.vector.tensor_scalar(
                                    rz, zps, rbsb[:, h : h + 1], 0.0,
                                    op0=OP.add, op1=OP.max,
                                )
                            rzs.append(rz)
                        for c4 in range(4):
                            nc.tensor.matmul(
                                ct, msgn[:, h, c4], rzs[c4 // 2][:, c4 % 2],
                                start=False, stop=(c4 == 3),
                            )
                        et = etp.tile([128, 512], f16)
                        nc.scalar.activation(et, ct, AF.Exp)
                        nc.tensor.matmul(
                            av[0:33], v2sb[:, kt, h], et,
                            start=(kt == 0), stop=(kt == 7),
                        )

                    if h > 0:
                        _finalize(h - 1, avs[h - 1])
                _finalize(H - 1, avs[H - 1])
            # ---------- phase C: O = O + relu(O @ Wo + bo) ----------
            with tc.tile_pool(name="ps_o", bufs=2, space="PSUM") as pso, \
                 tc.tile_pool(name="o_p", bufs=2) as op_:
                for j in range(4):
                    pso1 = pso.tile([128, 256], f32)
                    for dt_ in range(2):
                        nc.tensor.transpose(
                            pso1[:, ds(128 * dt_, 128)],
                            otsb[:, dt_, ds(128 * j, 128)],
                            ident32,
                        )
                    oj = op_.tile([128, 256], f32)
                    nc.vector.tensor_copy(oj, pso1)

                    pso2 = pso.tile([128, 256], f32)
                    for dt_ in range(2):
                        nc.tensor.matmul(
                            pso2, ot16[:, dt_, ds(128 * j, 128)], wos[:, dt_],
                            start=(dt_ == 0), stop=False,
                        )
                    nc.tensor.matmul(pso2, ones128r, bos, start=False, stop=True)
                    r2 = op_.tile([128, 256], f32)
                    nc.scalar.activation(r2, pso2, AF.Relu)
                    ofin = op_.tile([128, 256], f16)
                    nc.vector.tensor_add(ofin, oj, r2)
                    nc.sync.dma_start(d_o[ds(128 * j, 128), :], ofin)

    _split_multiwait(nc, mybir)
    return nc


def _split_multiwait(nc, mybir):
    """This walrus build only encodes ONE sem-wait per instruction; Tile's
    tail drain carries several. Split extras onto preceding NoOps."""
    for f in nc.m.functions:
        for blk in f.blocks:
            insts = list(blk.instructions)
            changed = False
            newlist = []
            for ins in insts:
                si = ins.sync_info
                if si is not None and len(si.on_wait) > 1:
                    waits = list(si.on_wait)
                    for j, w in enumerate(waits[:-1]):
                        newlist.append(
                            mybir.InstNoOp(
                                name=f"{ins.name}_splitw{j}",
                                engine=ins.engine,
                                ins=[],
                                outs=[],
                                sync_info=mybir.SyncInfo(on_wait=[w], on_update=[]),
                            )
                        )
                    ins.sync_info = mybir.SyncInfo(
                        on_wait=[waits[-1]], on_update=list(si.on_update)
                    )
                    changed = True
                newlist.append(ins)
            if changed:
                blk.instructions = newlist


def _host_constants(Wg1, bg1, wg2, bg2):
    """Compact folded layer-1 table, relu bias column, sign column."""
    aw = np.abs(wg2)  # [H, 3]
    sw = np.sign(wg2).astype(np.float32)
    kk = np.arange(32)

    # bdc[c, 4h+s] = |wg2[h,s]| * Wg1[h,c,s]
    bdc = np.zeros((3, 4 * H), np.float16)
    for c in range(3):
        for s in range(3):
            bdc[c, 4 * np.arange(H) + s] = (aw[:, s] * Wg1[:, c, s]).astype(np.float16)
    rb = np.zeros((128, H), np.float32)
    sgn = np.zeros((128, H), np.float32)
    for s in range(3):
        rb[4 * kk + s, :] = (aw[:, s] * bg1[:, s])[np.newaxis, :]
        sgn[4 * kk + s, :] = sw[np.newaxis, :, s]
    return bdc, rb, sgn


def _cast_threaded(src, dtype, nthreads=8):
    """dtype-cast in chunks on a thread pool (ml_dtypes casts release the
    GIL; harmless on a single-cpu host)."""
    from concurrent.futures import ThreadPoolExecutor

    flat = src.reshape(-1)
    out = np.empty(flat.shape, dtype)
    bounds = np.linspace(0, flat.size, nthreads + 1).astype(np.int64)

    def chunk(i):
        out[bounds[i]:bounds[i + 1]] = flat[bounds[i]:bounds[i + 1]]

    with ThreadPoolExecutor(nthreads) as ex:
        list(ex.map(chunk, range(nthreads)))
    return out.reshape(src.shape)


def kernel(**inputs):
    import ml_dtypes
    from concourse.bass_utils import run_bass_kernel_spmd

    Y = np.asarray(inputs["Y_lift"], np.float32)
    XP = np.asarray(inputs["X_pairs"], np.float32)
    PQ = np.asarray(inputs["presence_q"], np.float32)
    PK = np.asarray(inputs["presence_k"], np.float32)

    bdc, rb, sgn = _host_constants(
        np.asarray(inputs["Wg1"], np.float32),
        np.asarray(inputs["bg1"], np.float32),
        np.asarray(inputs["wg2"], np.float32),
        np.asarray(inputs["bg2"], np.float32),
    )

    # one bulk cast each; per-core slices below are zero-copy views
    XP8 = _cast_threaded(
        XP.reshape(B, N, 3 * N), ml_dtypes.float8_e3m4
    )
    Y16 = Y.astype(np.float16)
    w16 = np.concatenate(
        [
            np.asarray(inputs["Wq"], np.float32) / 16.0,
            np.asarray(inputs["Wk"], np.float32),
            np.asarray(inputs["Wv"], np.float32),
            np.asarray(inputs["Wo"], np.float32),
        ],
        axis=0,
    ).astype(np.float16)
    b16 = np.stack(
        [
            np.asarray(inputs["bq"], np.float32) / 16.0,
            np.asarray(inputs["bk"], np.float32),
            np.asarray(inputs["bv"], np.float32),
            np.asarray(inputs["bo"], np.float32),
        ],
        axis=0,
    ).astype(np.float16)

    if "nc" not in _CACHE:
        _CACHE["nc"] = _build_program()
    nc = _CACHE["nc"]

    shl = [np.zeros((128, 2), np.float32) for _ in range(2)]
    shl[0][:, 0] = 1.0
    shl[1][:, 1] = 1.0

    in_maps = []
    for core in range(NCORES):
        b, half = core // 2, core % 2
        rows = slice(half * R, half * R + R)
        pq2 = np.empty((2, R), np.float32)
        pq2[0] = PQ[b, rows]
        pq2[1] = 1.0 - PQ[b, rows]
        in_maps.append(
            {
                "yh": Y16[b, rows],
                "xp": XP8[b, rows],
                "wh": w16[128 * core : 128 * (core + 1)],
                "b": b16,
                "bdc": bdc,
                "rb": rb,
                "sgn": sgn,
                "pkc": np.ascontiguousarray(PK[b].reshape(8, 128).T),
                "pq2": pq2,
                "shl": shl[half],
            }
        )

    res = run_bass_kernel_spmd(nc, in_maps, core_ids=list(range(NCORES)))
    out = np.empty((B, N, D), np.float32)
    for core in range(NCORES):
        b, half = core // 2, core % 2
        out[b, half * R : half * R + R] = res.results[core]["o"]
    return out
